# revision 32
# baseline (speedup 1.0000x reference)
"""CollisionLoss kernel for Trainium2 (8 NeuronCores, Bass/Tile).

Computes: sum over (future, box) of masked AABB-overlap area between the
ego box (per-future, from the sdc trajectory) and 1M gt boxes per future,
times WEIGHT.

Distribution (memory-bound problem):
 - Host computes the 6 per-future ego AABBs (24 scalars) exactly as the
   reference does (O(1) work), replicated per partition, f32 + bf16 pair
   layouts.
 - future_gt_corners [6,1M,4,2] f32 (192 MB) and box_mask [6,1M] (6 MB)
   are sharded along the boxes axis across 8 cores (125000
   boxes/future/core) as zero-copy numpy views.
 - Each core streams its 24.75 MB once and emits 125 partial sums; the
   host adds 8x125 partials in float64.

Per-core layout: each future's [125000, 8]-float corner block is viewed
as [125 partitions, 1000 boxes], processed in SUB column subtiles.

Dataflow per subtile (all heavy ops on DVE; every operand unit-stride or
short-run contiguous -- strided APs measured 4-9x slower on HW):
  L1 (f32->bf16): max/min of the two 4-float half-boxes, box-major out.
  L2 (bf16):      combine pairs -> interleaved (x,y) AABB pair vectors.
  clamp/mask:     pm = min(pairs_hi, (xa1,ya1)) + maskbias;
                  qm = max(pairs_lo, (xa2,ya2)); negm = qm - pm
                  (ego pairs broadcast via step-0 APs; mask cast+bias on
                  ACT: {0 valid, -1e30 masked} duplicated per lane).
  ACT:            pos = relu(-negm) = (wpos, hpos) pairs (masked -> 0).
  area:           STT even*odd lanes with fused per-partition accum.
DMA: corner loads alternate SP/ACT issuers (two HW-DGE rings; note this
platform exposes only 5 SDMA engines shared by all queues, ~105 GB/s per
core -- the kernel runs at that DMA roofline, compute ~50% occupied).
"""

import numpy as np

DELTA = 0.5
WEIGHT = 1.0
W = 1.85 + DELTA
H = 4.084 + DELTA

F = 6
N = 1_000_000
CORES = 8
PER_CORE = N // CORES  # 125000
P = 125                # SBUF partitions used
BPR = PER_CORE // P    # boxes per partition row = 1000
SUB = 2                # column subtiles per future
B = BPR // SUB         # boxes per subtile column block

_prog = None
_last_in_maps = None


def _build_program(n_fut=F, p=P, bpr=BPR, sub=SUB, cbufs=4, l1bufs=3, sbufs=3, bf16=True, l1_dense=False):
    from contextlib import ExitStack

    import concourse.bacc as bacc
    import concourse.tile as tile
    from concourse import mybir

    Alu = mybir.AluOpType
    Act = mybir.ActivationFunctionType
    f32 = mybir.dt.float32
    u8 = mybir.dt.uint8
    mid = mybir.dt.bfloat16 if bf16 else f32

    b = bpr // sub
    nc = bacc.Bacc("TRN2", target_bir_lowering=False, debug=False)

    corners = [
        nc.dram_tensor(f"corners{f}", [p * bpr, 8], f32, kind="ExternalInput")
        for f in range(n_fut)
    ]
    masks = [
        nc.dram_tensor(f"mask{f}", [p * bpr], u8, kind="ExternalInput")
        for f in range(n_fut)
    ]
    ego = nc.dram_tensor("ego", [p, 4 * n_fut], f32, kind="ExternalInput")
    egob = nc.dram_tensor("egob", [p, 4 * n_fut], mybir.dt.bfloat16 if bf16 else f32, kind="ExternalInput")
    out = nc.dram_tensor("out", [p, 1], f32, kind="ExternalOutput")

    with tile.TileContext(nc) as tc, ExitStack() as ctx:
        const_pool = ctx.enter_context(tc.tile_pool(name="const", bufs=1))
        cpool = ctx.enter_context(tc.tile_pool(name="cd", bufs=cbufs))
        mpool = ctx.enter_context(tc.tile_pool(name="mask", bufs=2))
        l1pool = ctx.enter_context(tc.tile_pool(name="l1", bufs=l1bufs))
        spool = ctx.enter_context(tc.tile_pool(name="small", bufs=sbufs))

        ego_sb = const_pool.tile([p, 4 * n_fut], f32)
        nc.sync.dma_start(out=ego_sb[:], in_=ego.ap())
        egob_sb = const_pool.tile([p, 4 * n_fut], mid)
        nc.sync.dma_start(out=egob_sb[:], in_=egob.ap())
        ACC_W = n_fut * sub * 4
        acc = const_pool.tile([p, ACC_W], f32)

        # Heterogeneous schedule: big subtiles for bulk throughput, a
        # finely-split last future so the post-DMA compute drain is short.
        tiles = []
        for f in range(n_fut):
            if f == n_fut - 1 and bpr % (4 * sub) == 0:
                w = bpr // (4 * sub)
            else:
                w = bpr // sub
            for s0 in range(0, bpr, w):
                tiles.append((f, s0, w))
        n_tiles = len(tiles)
        assert n_tiles <= n_fut * sub * 4
        state = {}

        def ego_col(f, k):
            return ego_sb[:, 4 * f + k : 4 * f + k + 1]

        # DMA issue: the issuing sequencer is held for the whole transfer,
        # so one engine alone caps DMA throughput at transfer+setup per
        # period. SP takes most corner loads; ACT (which has compute slack)
        # takes every 6th plus the small mask loads, so transfers pack
        # back-to-back on the DMA engines.
        def s0_dma(t):
            f, s0, w = tiles[t]
            st = state[t] = {}
            cview = corners[f].ap().rearrange("(p b) c -> p (b c)", p=p)
            cd = cpool.tile([p, w * 8], f32, tag="cd")
            eng = nc.scalar if t % 6 == 0 else nc.sync
            eng.dma_start(out=cd[:], in_=cview[:, s0 * 8 : (s0 + w) * 8])
            st["cd"] = cd
            if s0 == 0:
                mview = masks[f].ap().rearrange("(p b) -> p b", p=p)
                mtile = mpool.tile([p, bpr], u8, tag="mask")
                nc.scalar.dma_start(out=mtile[:], in_=mview)
                state[("m", f)] = mtile

        def s1_l1(t):
            f, s0, w = tiles[t]
            st = state[t]
            cdh = st["cd"][:].rearrange("p (b h four) -> p b h four", h=2, four=4)
            # L1: one max + one min over the two 4-float half-boxes.
            # Output BOX-MAJOR [p, b, 4] (fully unit-stride writes):
            # per box: (M(x0,x2), M(y0,y2), M(x1,x3), M(y1,y3)).
            if l1_dense:
                wd = 8 * w - 2
                cdf = st["cd"][:]
                mx = l1pool.tile([p, 8 * w], mid, tag="mx")
                mn = l1pool.tile([p, 8 * w], mid, tag="mn")
                nc.vector.tensor_tensor(out=mx[:, 0:wd], in0=cdf[:, 0:wd],
                                        in1=cdf[:, 2 : 8 * w], op=Alu.max)
                nc.vector.tensor_tensor(out=mn[:, 0:wd], in0=cdf[:, 0:wd],
                                        in1=cdf[:, 2 : 8 * w], op=Alu.min)
            else:
                mx = l1pool.tile([p, 4 * w], mid, tag="mx")
                mn = l1pool.tile([p, 4 * w], mid, tag="mn")
                lo = cdh[:, :, 0, :]
                hi = cdh[:, :, 1, :]
                nc.vector.tensor_tensor(
                    out=mx[:].rearrange("p (b k) -> p b k", k=4), in0=lo, in1=hi,
                    op=Alu.max,
                )
                nc.vector.tensor_tensor(
                    out=mn[:].rearrange("p (b k) -> p b k", k=4), in0=lo, in1=hi,
                    op=Alu.min,
                )
            st["mx"], st["mn"] = mx, mn

        def s2_l2(t):
            f, s0, w = tiles[t]
            b = w
            st = state[t]
            if l1_dense:
                mxv = st["mx"][:].rearrange("p (b k) -> p b k", k=8)[:, :, 0:6]
                mnv = st["mn"][:].rearrange("p (b k) -> p b k", k=8)[:, :, 0:6]
                sel0, sel1 = (0, 2), (4, 6)
            else:
                mxv = st["mx"][:].rearrange("p (b k) -> p b k", k=4)
                mnv = st["mn"][:].rearrange("p (b k) -> p b k", k=4)
                sel0, sel1 = (0, 2), (2, 4)
            # L2 -> interleaved (x, y) pair vectors [p, 2b], contiguous.
            xy1 = spool.tile([p, 2 * b], mid, tag="xy1")  # (xb1, yb1) pairs
            xy2 = spool.tile([p, 2 * b], mid, tag="xy2")  # (xb2, yb2) pairs
            nc.vector.tensor_tensor(
                out=xy1[:].rearrange("p (b two) -> p b two", two=2),
                in0=mxv[:, :, sel0[0]:sel0[1]], in1=mxv[:, :, sel1[0]:sel1[1]], op=Alu.max,
            )
            nc.vector.tensor_tensor(
                out=xy2[:].rearrange("p (b two) -> p b two", two=2),
                in0=mnv[:, :, sel0[0]:sel0[1]], in1=mnv[:, :, sel1[0]:sel1[1]], op=Alu.min,
            )
            # mask -> {0 valid, -1e30 masked}, duplicated per (x,y) lane
            maskm = spool.tile([p, 2 * b], mid, tag="maskm")
            msrc = state[("m", f)][:, s0 : s0 + w]
            nc.scalar.activation(
                out=maskm[:].rearrange("p (b two) -> p b two", two=2),
                in_=msrc.rearrange("p (b one) -> p b one", one=1).broadcast_to((p, b, 2)),
                func=Act.Copy, bias=-1e30, scale=1e30,
            )
            st.update(xy1=xy1, xy2=xy2, maskm=maskm)

        def s3(t):
            f, s0, w = tiles[t]
            b = w
            st = state[t]
            ehi = egob_sb[:, 4 * f : 4 * f + 2].rearrange(
                "p (one two) -> p one two", one=1).broadcast_to((p, b, 2))
            # pm = min((xb1,yb1), (xa1,ya1)) ; pmm = pm + maskm
            pm = spool.tile([p, 2 * b], mid, tag="pm")
            nc.vector.tensor_tensor(
                out=pm[:].rearrange("p (b two) -> p b two", two=2),
                in0=st["xy1"][:].rearrange("p (b two) -> p b two", two=2),
                in1=ehi, op=Alu.min,
            )
            pmm = spool.tile([p, 2 * b], mid, tag="pmm")
            nc.vector.tensor_tensor(out=pmm[:], in0=pm[:], in1=st["maskm"][:],
                                    op=Alu.add)
            st["pmm"] = pmm

        def s4(t):
            f, s0, w = tiles[t]
            b = w
            st = state[t]
            elo = egob_sb[:, 4 * f + 2 : 4 * f + 4].rearrange(
                "p (one two) -> p one two", one=1).broadcast_to((p, b, 2))
            qm = spool.tile([p, 2 * b], mid, tag="qm")
            nc.vector.tensor_tensor(
                out=qm[:].rearrange("p (b two) -> p b two", two=2),
                in0=st["xy2"][:].rearrange("p (b two) -> p b two", two=2),
                in1=elo, op=Alu.max,
            )
            negm = spool.tile([p, 2 * b], mid, tag="negm")
            nc.vector.tensor_tensor(out=negm[:], in0=qm[:], in1=st["pmm"][:],
                                    op=Alu.subtract)
            st["negm"] = negm

        def s5(t):
            f, s0, w = tiles[t]
            b = w
            st = state[t]
            # pos = relu(-negm) = (wpos_masked, hpos) interleaved
            pos = spool.tile([p, 2 * b], mid, tag="pos")
            nc.scalar.activation(out=pos[:], in_=st["negm"][:], func=Act.Relu,
                                 scale=-1.0)
            st["pos"] = pos

        def s6(t):
            f, s0, w = tiles[t]
            b = w
            st = state[t]
            # area = wpos * hpos (even * odd lanes), accumulated per
            # partition into acc column.
            posv = st["pos"][:].rearrange("p (b two) -> p b two", two=2)
            scr = spool.tile([p, b], mid, tag="scr")
            nc.vector.scalar_tensor_tensor(
                out=scr[:], in0=posv[:, :, 0], scalar=0.0, in1=posv[:, :, 1],
                op0=Alu.bypass, op1=Alu.mult,
                accum_out=acc[:, t : t + 1],
            )
            del state[t]

        # 7-stage software pipeline: every cross-engine hop of the tail
        # chain lands in its own period, so no in-order engine queue ever
        # blocks on a same-subtile dependency.
        stages = [s0_dma, s1_l1, s2_l2, s3, s4, s5, s6]
        for t in range(n_tiles + len(stages) - 1):
            for k, fn in enumerate(stages):
                tt = t - k
                if 0 <= tt < n_tiles:
                    fn(tt)

        total = const_pool.tile([p, 1], f32)
        nc.vector.reduce_sum(out=total[:], in_=acc[:, 0:n_tiles], axis=mybir.AxisListType.X)
        nc.sync.dma_start(out=out.ap(), in_=total[:])

    nc.compile()
    return nc


def _get_prog():
    global _prog
    if _prog is None:
        _prog = _build_program()
    return _prog


def _ego_aabb(sdc_traj_all, sdc_planning_gt):
    """Per-future ego AABB [F,4] = (xa1, xa2, ya1, ya2), mirroring reference."""
    sdc_traj_all = np.asarray(sdc_traj_all, dtype=np.float32)
    sdc_planning_gt = np.asarray(sdc_planning_gt, dtype=np.float32)
    x = sdc_traj_all[0, :, 0]
    y = sdc_traj_all[0, :, 1]
    theta = sdc_planning_gt[0, :, 2]
    local = np.array(
        [[W / 2, -H / 2], [W / 2, H / 2], [-W / 2, H / 2], [-W / 2, -H / 2]],
        dtype=np.float32,
    )
    c, s = np.cos(theta), np.sin(theta)
    rot = np.stack([np.stack([c, s], -1), np.stack([-s, c], -1)], -2)  # [F,2,2]
    corners = np.einsum("fij,kj->fki", rot, local) + np.stack([x, y], -1)[:, None, :]
    corners = corners.astype(np.float32)
    xa1 = corners[..., 0].max(-1)
    ya1 = corners[..., 1].max(-1)
    xa2 = corners[..., 0].min(-1)
    ya2 = corners[..., 1].min(-1)
    return np.stack([xa1, xa2, ya1, ya2], -1).astype(np.float32)  # [F,4]


def kernel(sdc_traj_all, sdc_planning_gt, sdc_planning_gt_mask, future_gt_corners, box_mask):
    from concourse.bass_utils import run_bass_kernel_spmd

    corners = np.asarray(future_gt_corners, dtype=np.float32).reshape(F, N, 8)
    mask = np.asarray(box_mask)
    if mask.dtype == np.bool_:
        mask_u8 = mask.view(np.uint8)
    else:
        mask_u8 = (mask != 0).astype(np.uint8)

    eg = _ego_aabb(sdc_traj_all, sdc_planning_gt)  # [F,4] = (xa1, xa2, ya1, ya2)
    ego_arr = np.ascontiguousarray(
        np.broadcast_to(eg.reshape(4 * F), (P, 4 * F)), dtype=np.float32
    )
    import ml_dtypes
    # pair layout per future: (xa1, ya1, xa2, ya2)
    egp = np.stack([eg[:, 0], eg[:, 2], eg[:, 1], eg[:, 3]], -1).reshape(4 * F)
    egob_arr = np.ascontiguousarray(
        np.broadcast_to(egp, (P, 4 * F))
    ).astype(ml_dtypes.bfloat16)

    in_maps = []
    for cidx in range(CORES):
        lo, hi = cidx * PER_CORE, (cidx + 1) * PER_CORE
        m = {"ego": ego_arr, "egob": egob_arr}
        for f in range(F):
            m[f"corners{f}"] = corners[f, lo:hi]
            m[f"mask{f}"] = mask_u8[f, lo:hi]
        in_maps.append(m)

    global _last_in_maps
    _last_in_maps = in_maps
    res = run_bass_kernel_spmd(_get_prog(), in_maps, list(range(CORES))).results
    total = 0.0
    for r in res:
        total += float(r["out"].astype(np.float64).sum())
    return np.array([total], dtype=np.float32) * np.float32(WEIGHT)
